# revision 1
# baseline (speedup 1.0000x reference)
"""Trainium2 Bass kernel for nn_PoolHiddenNet (gnn_message_passing).

Math (per scene of N=32 peds, uniform S=64 scenes, B=2048):
  rel[j,k]  = pos[k] - pos[j]
  x[j,k]    = [rel @ W_emb + b_emb, h[k]]
  y1        = relu(BN1(x @ W1 + b1))          per-scene BN over N*N rows
  z         = y1 @ W2 + b2
  out[j]    = max_k relu(BN2(z))[j,k]

Key algebraic restructuring used here (validated vs the jax reference to
~5e-6 scaled error in fp32):
  * Layer 1 is rank-structured: (x @ W1)[j,k] = a[k] - c[j] + const, with
    a = [h, pos] @ [W1h; W1e], c = pos @ W1e, W1e = W_emb @ W1[:64].
    This turns a 65536x128x512 matmul into a 2048x66x512 one.
  * Training-mode BN is invariant to constant row shifts, so b_emb/b1/b2
    drop out entirely.
  * BN1 stats over the (j,k) product set decompose exactly:
    mean = mean(a) - mean(c), var = var(a) + var(c).
  * BN2's affine+relu is monotone (g2 > 0), so the max over k is taken on
    raw z and the affine+relu applied to the pooled [32, 1024] result.
  * BN2 mean comes from an extra tiny matmul W2^T @ rowsum(y1) (rowsum is a
    free accumulator output of the relu pass); var from E[z^2]-E[z]^2 where
    E[z^2] uses Square-with-accumulate passes over PSUM.

Sharding: data-parallel over scenes, 8 scenes per NeuronCore, weights
replicated. Matmuls run as float32r (full PE rate); everything else fp32.
"""

import sys

sys.path.insert(0, "/opt/trn_rl_repo")

import numpy as np

import concourse.bacc as bacc
import concourse.bass as bass
import concourse.mybir as mybir
import concourse.tile as tile
from concourse import masks
from concourse.bass_utils import run_bass_kernel_spmd

F32 = mybir.dt.float32
F32R = mybir.dt.float32r
AX = mybir.AxisListType
OP = mybir.AluOpType
AF = mybir.ActivationFunctionType

NCORES = 8
S, N, B = 64, 32, 2048
E, H, D1, D2 = 64, 64, 512, 1024
SC = S // NCORES          # scenes per core
ROWS = SC * N             # batch rows per core
FT1 = D1 // 128           # layer-1 feature tiles (4)
MT2 = D2 // 128           # layer-2 feature tiles (8)
EPS = 1e-5
SUBS_ON_DVE = 3  # how many of the 4 y1-sub builds run on DVE vs POOL
RELUS_ON_DVE = 1  # how many relus run as DVE ts pairs (2x SBUF mode) vs ACT


def _build_kernel(nc: bass.Bass, reps: int = 1):
    h_ap = nc.dram_tensor("h", [ROWS, H], F32, kind="ExternalInput").ap()
    pos_ap = nc.dram_tensor("pos", [ROWS, 2], F32, kind="ExternalInput").ap()
    wcat_ap = nc.dram_tensor("wcat", [H + 2, D1], F32, kind="ExternalInput").ap()
    w2_ap = nc.dram_tensor("w2", [D1, D2], F32, kind="ExternalInput").ap()
    g1_ap = nc.dram_tensor("g1", [128, FT1], F32, kind="ExternalInput").ap()
    beta1_ap = nc.dram_tensor("beta1", [128, FT1], F32, kind="ExternalInput").ap()
    g2_ap = nc.dram_tensor("g2", [128, MT2], F32, kind="ExternalInput").ap()
    beta2_ap = nc.dram_tensor("beta2", [128, MT2], F32, kind="ExternalInput").ap()
    out_ap = nc.dram_tensor("out", [ROWS, D2], F32, kind="ExternalOutput").ap()

    with tile.TileContext(nc) as tc:
        for _ in range(reps):
            _emit(tc, h_ap, pos_ap, wcat_ap, w2_ap, g1_ap, beta1_ap, g2_ap, beta2_ap, out_ap)


def _emit(tc, h_ap, pos_ap, wcat_ap, w2_ap, g1_ap, beta1_ap, g2_ap, beta2_ap, out_ap):
    nc = tc.nc
    import contextlib

    ctx = contextlib.ExitStack()
    with ctx:
        const = ctx.enter_context(tc.tile_pool(name="const", bufs=1))
        bn1p = ctx.enter_context(tc.tile_pool(name="bn1", bufs=1))
        y1p = ctx.enter_context(tc.tile_pool(name="y1", bufs=4))
        smallp = ctx.enter_context(tc.tile_pool(name="small", bufs=4))
        sqp = ctx.enter_context(tc.tile_pool(name="sq", bufs=3))
        statp = ctx.enter_context(tc.tile_pool(name="stat", bufs=2))
        outp = ctx.enter_context(tc.tile_pool(name="ostage", bufs=4))
        zpool = ctx.enter_context(tc.tile_pool(name="zp", bufs=3, space="PSUM"))
        meanp = ctx.enter_context(tc.tile_pool(name="meanp", bufs=2, space="PSUM"))

        # ---- constants / weights ----
        ident = const.tile([128, 128], F32)
        masks.make_identity(nc, ident[:])
        eps_t = const.tile([128, 1], F32)
        nc.gpsimd.memset(eps_t[:], EPS)

        # small/latency-critical DMAs first — the 2 MB w2 load goes last so it
        # doesn't head-of-line block the inputs the preamble needs
        wcat_sb = const.tile([H + 2, D1], F32)          # rows 0:64 = W1h, 64:66 = W1e
        nc.sync.dma_start(wcat_sb[:], wcat_ap)
        g1_sb = const.tile([128, FT1], F32)
        nc.sync.dma_start(g1_sb[:], g1_ap)
        beta1_sb = const.tile([128, FT1], F32)
        nc.sync.dma_start(beta1_sb[:], beta1_ap)
        g2_sb = const.tile([128, MT2], F32)
        nc.sync.dma_start(g2_sb[:], g2_ap)
        beta2_sb = const.tile([128, MT2], F32)
        nc.sync.dma_start(beta2_sb[:], beta2_ap)

        # ---- build xsT [66, 256] = [h; pos]^T via PE transpose ----
        xsT = const.tile([H + 2, ROWS], F32)
        for half in range(2):
            hp = const.tile([128, H + 2], F32, tag="hp", bufs=2)
            rs = slice(half * 128, (half + 1) * 128)
            nc.sync.dma_start(hp[:, 0:H], h_ap[rs, :])
            nc.sync.dma_start(hp[:, H : H + 2], pos_ap[rs, :])
            tp = zpool.tile([H + 2, 128], F32, tag="z")
            nc.tensor.transpose(tp[:], hp[:], ident[:])
            nc.scalar.copy(xsT[:, half * 128 : (half + 1) * 128], tp[:])

        w2_sb = const.tile([128, FT1 * D2], F32R)       # [p, kt*D2 + f]
        nc.sync.dma_start(
            w2_sb[:].rearrange("p (kt f) -> p kt f", kt=FT1),
            w2_ap.bitcast(F32R).rearrange("(kt p) f -> p kt f", p=128),
        )
        w2v = w2_sb[:].rearrange("p (kt f) -> p kt f", kt=FT1)

        # ---- layer 1: aT, cT  [128, ft*256 + s*32 + k] ----
        a_sb = const.tile([128, FT1 * ROWS], F32)
        c_sb = const.tile([128, FT1 * ROWS], F32)
        for ft in range(FT1):
            fs = slice(ft * ROWS, (ft + 1) * ROWS)
            apz = zpool.tile([128, ROWS], F32, tag="z")
            nc.tensor.matmul(
                apz[:],
                lhsT=wcat_sb[:, ft * 128 : (ft + 1) * 128],
                rhs=xsT[:],
                start=True,
                stop=True,
            )
            nc.scalar.copy(a_sb[:, fs], apz[:])
            cpz = zpool.tile([128, ROWS], F32, tag="z")
            nc.tensor.matmul(
                cpz[:],
                lhsT=wcat_sb[H : H + 2, ft * 128 : (ft + 1) * 128],
                rhs=xsT[H : H + 2, :],
                start=True,
                stop=True,
            )
            nc.scalar.copy(c_sb[:, fs], cpz[:])

        # ---- BN1 stats: per (feature, scene) over the 32 peds ----
        # grp = ft*SC + s  (32 groups)
        NG = FT1 * SC
        suma = bn1p.tile([128, NG], F32)
        sumc = bn1p.tile([128, NG], F32)
        sqa = bn1p.tile([128, NG], F32)
        sqc = bn1p.tile([128, NG], F32)
        scr = bn1p.tile([128, FT1 * ROWS], F32, tag="bn1scr", bufs=2)
        scr2 = bn1p.tile([128, FT1 * ROWS], F32, tag="bn1scr", bufs=2)
        a3 = a_sb[:].rearrange("p (g k) -> p g k", k=N)
        c3 = c_sb[:].rearrange("p (g k) -> p g k", k=N)

        # fast path: scene-0 stats first, so relu(scene 0) — and with it the
        # first layer-2 matmul — doesn't wait for the full 8-scene stats chain
        a0v = a_sb[:].rearrange("p (ft s k) -> p ft s k", s=SC, k=N)[:, :, 0, :]
        c0v = c_sb[:].rearrange("p (ft s k) -> p ft s k", s=SC, k=N)[:, :, 0, :]
        suma0 = bn1p.tile([128, FT1], F32)
        sumc0 = bn1p.tile([128, FT1], F32)
        sqa0 = bn1p.tile([128, FT1], F32)
        sqc0 = bn1p.tile([128, FT1], F32)
        scr0 = bn1p.tile([128, 2 * FT1 * N], F32)
        nc.vector.tensor_reduce(out=suma0[:], in_=a0v, axis=AX.X, op=OP.add)
        nc.vector.tensor_reduce(out=sumc0[:], in_=c0v, axis=AX.X, op=OP.add)
        s0a = scr0[:, 0 : FT1 * N].rearrange("p (ft k) -> p ft k", k=N)
        s0c = scr0[:, FT1 * N :].rearrange("p (ft k) -> p ft k", k=N)
        nc.scalar.activation(out=s0a, in_=a0v, func=AF.Square)
        nc.scalar.activation(out=s0c, in_=c0v, func=AF.Square)
        nc.vector.tensor_reduce(out=sqa0[:], in_=s0a, axis=AX.X, op=OP.add)
        nc.vector.tensor_reduce(out=sqc0[:], in_=s0c, axis=AX.X, op=OP.add)
        m0a = bn1p.tile([128, FT1], F32)
        m0c = bn1p.tile([128, FT1], F32)
        v0 = bn1p.tile([128, FT1], F32)
        t0t = bn1p.tile([128, FT1], F32)
        nc.vector.tensor_scalar(m0a[:], suma0[:], 1.0 / N, None, OP.mult)
        nc.vector.tensor_scalar(m0c[:], sumc0[:], 1.0 / N, None, OP.mult)
        # v0 = (sqa0 + sqc0)/N - m0a^2 - m0c^2
        nc.vector.tensor_tensor(out=v0[:], in0=sqa0[:], in1=sqc0[:], op=OP.add)
        nc.vector.tensor_scalar(v0[:], v0[:], 1.0 / N, None, OP.mult)
        nc.vector.tensor_tensor(out=t0t[:], in0=m0a[:], in1=m0a[:], op=OP.mult)
        nc.vector.tensor_tensor(out=v0[:], in0=v0[:], in1=t0t[:], op=OP.subtract)
        nc.vector.tensor_tensor(out=t0t[:], in0=m0c[:], in1=m0c[:], op=OP.mult)
        nc.vector.tensor_tensor(out=v0[:], in0=v0[:], in1=t0t[:], op=OP.subtract)
        sd0 = bn1p.tile([128, FT1], F32)
        nc.scalar.activation(out=sd0[:], in_=v0[:], func=AF.Sqrt, bias=eps_t[:], scale=1.0)
        inv0 = bn1p.tile([128, FT1], F32)
        nc.vector.reciprocal(out=inv0[:], in_=sd0[:])
        s1_0 = bn1p.tile([128, FT1], F32)
        t1_0 = bn1p.tile([128, FT1], F32)
        nc.vector.tensor_tensor(out=s1_0[:], in0=inv0[:], in1=g1_sb[:], op=OP.mult)
        nc.vector.tensor_tensor(out=t1_0[:], in0=m0a[:], in1=m0c[:], op=OP.subtract)
        nc.vector.tensor_tensor(out=t1_0[:], in0=t1_0[:], in1=s1_0[:], op=OP.mult)
        nc.vector.tensor_tensor(out=t1_0[:], in0=beta1_sb[:], in1=t1_0[:], op=OP.subtract)

        s1 = bn1p.tile([128, NG], F32)
        t1 = bn1p.tile([128, NG], F32)

        def emit_bn1_full():
            # full 8-scene BN1 stats; emitted AFTER scene 0's y1 build so the
            # first layer-2 matmuls don't queue behind this chain on DVE/ACT
            nc.vector.tensor_reduce(out=suma[:], in_=a3, axis=AX.X, op=OP.add)
            nc.vector.tensor_reduce(out=sumc[:], in_=c3, axis=AX.X, op=OP.add)
            nc.scalar.square(out=scr[:], in_=a_sb[:])
            nc.vector.tensor_reduce(
                out=sqa[:], in_=scr[:].rearrange("p (g k) -> p g k", k=N), axis=AX.X, op=OP.add
            )
            nc.scalar.square(out=scr2[:], in_=c_sb[:])
            nc.vector.tensor_reduce(
                out=sqc[:], in_=scr2[:].rearrange("p (g k) -> p g k", k=N), axis=AX.X, op=OP.add
            )
            ma = bn1p.tile([128, NG], F32)
            mc = bn1p.tile([128, NG], F32)
            va = bn1p.tile([128, NG], F32)
            vc = bn1p.tile([128, NG], F32)
            tmp1 = bn1p.tile([128, NG], F32)
            nc.vector.tensor_scalar(ma[:], suma[:], 1.0 / N, None, OP.mult)
            nc.vector.tensor_scalar(mc[:], sumc[:], 1.0 / N, None, OP.mult)
            # va = sqa/N - ma^2
            nc.vector.tensor_tensor(out=tmp1[:], in0=ma[:], in1=ma[:], op=OP.mult)
            nc.vector.tensor_scalar(va[:], sqa[:], 1.0 / N, None, OP.mult)
            nc.vector.tensor_tensor(out=va[:], in0=va[:], in1=tmp1[:], op=OP.subtract)
            nc.vector.tensor_tensor(out=tmp1[:], in0=mc[:], in1=mc[:], op=OP.mult)
            nc.vector.tensor_scalar(vc[:], sqc[:], 1.0 / N, None, OP.mult)
            nc.vector.tensor_tensor(out=vc[:], in0=vc[:], in1=tmp1[:], op=OP.subtract)
            var1 = bn1p.tile([128, NG], F32)
            nc.vector.tensor_tensor(out=var1[:], in0=va[:], in1=vc[:], op=OP.add)
            sd1 = bn1p.tile([128, NG], F32)
            nc.scalar.activation(out=sd1[:], in_=var1[:], func=AF.Sqrt, bias=eps_t[:], scale=1.0)
            inv1 = bn1p.tile([128, NG], F32)
            nc.vector.reciprocal(out=inv1[:], in_=sd1[:])
            m1 = bn1p.tile([128, NG], F32)
            nc.vector.tensor_tensor(out=m1[:], in0=ma[:], in1=mc[:], op=OP.subtract)
            for ft in range(FT1):
                gs = slice(ft * SC, (ft + 1) * SC)
                nc.vector.tensor_scalar(
                    s1[:, gs], inv1[:, gs], g1_sb[:, ft : ft + 1], None, OP.mult
                )
            nc.vector.tensor_tensor(out=m1[:], in0=m1[:], in1=s1[:], op=OP.mult)
            for ft in range(FT1):
                gs = slice(ft * SC, (ft + 1) * SC)
                nc.vector.tensor_scalar(
                    t1[:, gs], m1[:, gs], -1.0, beta1_sb[:, ft : ft + 1], OP.mult, OP.add
                )

        # ---- per-scene pipeline ----
        # Four-part software pipeline per scene:
        #   A1(t): build y1[t] (POOL sub + ACT relu) — emitted one scene EARLY so
        #          the in-order POOL/ACT queues prepare scene t during the PE
        #          matmuls of scene t-1.
        #   B1(t): BN2 stat finalize + pooled affine (POOL/ACT only, tiny ops).
        #   A2(t): the PE matmuls + ACT square + DVE maxpool/mean gather.
        #   B2(t): PE transposes + DVE PSUM->SBUF copy + output DMA.
        def emit_A1(s):
            # y1[ft][p, j*32+k] = relu((a[p,k] - c[p,j]) * s1 + t1), rowsum -> u
            # u holds rowsum(y1) in even columns; odd columns are zero padding so
            # the fp32r mean-matmul gets an even moving free dim (ISA requirement)
            u = smallp.tile([128, FT1 * 2], F32R, tag="u")
            nc.gpsimd.memset(u[:].bitcast(mybir.dt.uint32), 0)
            y1 = []
            for ft in range(FT1):
                yt = y1p.tile([128, N * N], F32R, tag=f"y1_{ft}")
                acol = a_sb[:, ft * ROWS + s * N : ft * ROWS + (s + 1) * N]
                ccol = c_sb[:, ft * ROWS + s * N : ft * ROWS + (s + 1) * N]
                eng = nc.vector if ft < SUBS_ON_DVE else nc.gpsimd
                eng.tensor_tensor(
                    out=yt[:].rearrange("p (j k) -> p j k", k=N),
                    in0=acol.unsqueeze(1).broadcast_to([128, N, N]),
                    in1=ccol.unsqueeze(2).broadcast_to([128, N, N]),
                    op=OP.subtract,
                )
                if s == 0:
                    sc_ap, bi_ap = s1_0[:, ft : ft + 1], t1_0[:, ft : ft + 1]
                else:
                    g = ft * SC + s
                    sc_ap, bi_ap = s1[:, g : g + 1], t1[:, g : g + 1]
                if ft < RELUS_ON_DVE:
                    # relu for ft0 as a fused DVE ts pair — tensor_scalar gets
                    # the 2x SBUF perf mode, halving the cost vs TT/activation
                    nc.vector.tensor_scalar(yt[:], yt[:], sc_ap, bi_ap, OP.mult, OP.add)
                    with nc.allow_low_precision(reason="f32r accum is fp32 width"):
                        nc.vector.tensor_scalar(
                            yt[:], yt[:], 0.0, 0.0, OP.max, OP.add,
                            accum_out=u[:, 2 * ft : 2 * ft + 1],
                        )
                else:
                    with nc.allow_low_precision(reason="f32r accum is fp32 width"):
                        nc.scalar.activation(
                            out=yt[:],
                            in_=yt[:],
                            func=AF.Relu,
                            scale=sc_ap,
                            bias=bi_ap,
                            accum_out=u[:, 2 * ft : 2 * ft + 1],
                        )
                y1.append(yt)
            return u, y1

        def emit_A2(s, u, y1):
            q = smallp.tile([128, MT2], F32, tag="q")
            pooled = smallp.tile([128, MT2 * N], F32, tag="pooled")
            meanz = smallp.tile([128, MT2], F32, tag="meanz")
            mean_ps = meanp.tile([128, MT2 * 2], F32, tag="meanps")
            for m in range(MT2):
                ms = slice(m * 128, (m + 1) * 128)
                zp = zpool.tile([128, N * N], F32, tag="z")
                for kt in range(FT1):
                    for ch in range(2):
                        cs = slice(ch * 512, (ch + 1) * 512)
                        nc.tensor.matmul(
                            zp[:, cs],
                            lhsT=w2v[:, kt, ms],
                            rhs=y1[kt][:, cs],
                            start=(kt == 0),
                            stop=(kt == FT1 - 1),
                        )
                # E[z^2] accumulator (one ACT square pass over the full PSUM tile)
                sq = sqp.tile([128, N * N], F32, tag="sqscr")
                nc.scalar.activation(
                    out=sq[:],
                    in_=zp[:],
                    func=AF.Square,
                    accum_out=q[:, m : m + 1],
                )
                # max over k: DVE segmented reduce straight from PSUM
                nc.vector.tensor_reduce(
                    out=pooled[:, m * N : (m + 1) * N],
                    in_=zp[:].rearrange("p (j k) -> p j k", k=N),
                    axis=AX.X,
                    op=OP.max,
                )
            # column mean of z via W2^T @ rowsum(y1) — emitted AFTER the whole
            # z block so PE never waits on the relu accumulators (u columns),
            # which only complete a few microseconds into the scene period
            for m in range(MT2):
                ms = slice(m * 128, (m + 1) * 128)
                for kt in range(FT1):
                    nc.tensor.matmul(
                        mean_ps[:, 2 * m : 2 * m + 2],
                        lhsT=w2v[:, kt, ms],
                        rhs=u[:, 2 * kt : 2 * kt + 2],
                        start=(kt == 0),
                        stop=(kt == FT1 - 1),
                    )
            return q, pooled, meanz, mean_ps

        def emit_B1(s, q, pooled, meanz, mean_ps):
            # gather the PSUM column means one period later — by now the
            # mean-matmuls finished long ago, so DVE never head-of-line stalls
            nc.vector.tensor_scalar(
                meanz[:].unsqueeze(2),
                mean_ps[:].rearrange("p (m t) -> p m t", t=2)[:, :, 0:1],
                1.0 / (N * N),
                None,
                OP.mult,
            )
            # BN2 stat finalize on POOL (+ one ACT sqrt); all inputs are SBUF
            varz = statp.tile([128, MT2], F32, tag="varz")
            mz2 = statp.tile([128, MT2], F32, tag="mz2")
            nc.gpsimd.tensor_tensor(out=mz2[:], in0=meanz[:], in1=meanz[:], op=OP.mult)
            nc.gpsimd.tensor_scalar(varz[:], q[:], 1.0 / (N * N), None, OP.mult)
            nc.gpsimd.tensor_tensor(out=varz[:], in0=varz[:], in1=mz2[:], op=OP.subtract)
            sd2 = statp.tile([128, MT2], F32, tag="sd2")
            nc.scalar.activation(out=sd2[:], in_=varz[:], func=AF.Sqrt, bias=eps_t[:], scale=1.0)
            s2 = statp.tile([128, MT2], F32, tag="s2")
            t2 = statp.tile([128, MT2], F32, tag="t2")
            inv2 = statp.tile([128, MT2], F32, tag="inv2")
            nc.vector.reciprocal(out=inv2[:], in_=sd2[:])
            nc.gpsimd.tensor_tensor(out=s2[:], in0=g2_sb[:], in1=inv2[:], op=OP.mult)
            nc.gpsimd.tensor_tensor(out=t2[:], in0=meanz[:], in1=s2[:], op=OP.mult)
            nc.gpsimd.tensor_tensor(out=t2[:], in0=beta2_sb[:], in1=t2[:], op=OP.subtract)
            # pooled affine + relu (POOL): 3 full-width ops with per-m scale and
            # shift broadcast along the ped axis, instead of 16 tiny per-m ops
            p3 = pooled[:].rearrange("p (m j) -> p m j", j=N)
            nc.gpsimd.tensor_tensor(
                out=p3, in0=p3, in1=s2[:].unsqueeze(2).broadcast_to([128, MT2, N]),
                op=OP.mult,
            )
            nc.gpsimd.tensor_tensor(
                out=p3, in0=p3, in1=t2[:].unsqueeze(2).broadcast_to([128, MT2, N]),
                op=OP.add,
            )
            nc.gpsimd.tensor_scalar(pooled[:], pooled[:], 0.0, None, OP.max)

        def emit_B2(s, pooled):
            # 32x32 block transpose on DVE: outSBT[bp*32+j, m*32+q] =
            # pooled[bp*32+q, m*32+j] = feature (m*128+bp*32+q) of ped j.
            outSBT = outp.tile([128, MT2 * N], F32, tag="outSBT")
            # one StreamTranspose covers all 4x8 32x32 blocks in place-position
            nc.vector.transpose(out=outSBT[:], in_=pooled[:])
            dst = out_ap[s * N : (s + 1) * N, :].rearrange(
                "j (m b qq) -> j b m qq", b=4, qq=32
            )
            for bp in range(4):
                pr = slice(bp * 32, (bp + 1) * 32)
                nc.sync.dma_start(
                    dst[:, bp, :, :],
                    outSBT[pr, :].rearrange("p (m qq) -> p m qq", qq=32),
                )

        # pipeline order per iteration: B1(s-1) first (tiny stat ops — their
        # consumers sqrt/affine must not queue behind the next sub block),
        # then A1(s+1) (prepare y1 a scene ahead), then A2(s) (matmuls), then
        # B2(s-1) (output tail, after the PE matmul block).
        prep = {0: emit_A1(0)}
        emit_bn1_full()
        st = {}
        for s in range(SC):
            if s - 1 in st:
                emit_B1(s - 1, *st[s - 1])
            if s + 1 < SC:
                prep[s + 1] = emit_A1(s + 1)
            st[s] = emit_A2(s, *prep.pop(s))
            if s - 1 in st:
                emit_B2(s - 1, st.pop(s - 1)[1])
        last = st.pop(SC - 1)
        emit_B1(SC - 1, *last)
        emit_B2(SC - 1, last[1])


_CACHED = None


def _get_nc():
    global _CACHED
    if _CACHED is None:
        nc = bacc.Bacc("TRN2", target_bir_lowering=False, debug=False)
        _build_kernel(nc)
        nc.compile()
        _CACHED = nc
    return _CACHED


def _make_in_maps(inputs):
    h2 = np.ascontiguousarray(inputs["h_states"].reshape(B, H), dtype=np.float32)
    pos = np.ascontiguousarray(inputs["end_pos"], dtype=np.float32)
    W_emb = np.asarray(inputs["W_emb"], dtype=np.float32)
    W1 = np.asarray(inputs["W1"], dtype=np.float32)
    W2 = np.ascontiguousarray(inputs["W2"], dtype=np.float32)
    W1e = (W_emb.astype(np.float64) @ W1[:E].astype(np.float64)).astype(np.float32)
    Wcat = np.ascontiguousarray(np.concatenate([W1[E:], W1e], axis=0))  # [W1h; W1e]

    def pftile(v, nt):
        return np.ascontiguousarray(np.asarray(v, np.float32).reshape(nt, 128).T)

    g1m = pftile(inputs["g1"], FT1)
    beta1m = pftile(inputs["beta1"], FT1)
    g2m = pftile(inputs["g2"], MT2)
    beta2m = pftile(inputs["beta2"], MT2)

    in_maps = []
    for c in range(NCORES):
        sl = slice(c * ROWS, (c + 1) * ROWS)
        in_maps.append(
            {
                "h": np.ascontiguousarray(h2[sl]),
                "pos": np.ascontiguousarray(pos[sl]),
                "wcat": Wcat,
                "w2": W2,
                "g1": g1m,
                "beta1": beta1m,
                "g2": g2m,
                "beta2": beta2m,
            }
        )
    return in_maps


def kernel(**inputs) -> np.ndarray:
    nc = _get_nc()
    in_maps = _make_in_maps(inputs)
    res = run_bass_kernel_spmd(nc, in_maps, core_ids=list(range(NCORES)))
    return np.concatenate([r["out"] for r in res.results], axis=0).astype(np.float32)


def kernel_profiled(inputs, **kw):
    nc = _get_nc()
    in_maps = _make_in_maps(inputs)
    res = run_bass_kernel_spmd(nc, in_maps, core_ids=list(range(NCORES)), **kw)
    out = np.concatenate([r["out"] for r in res.results], axis=0).astype(np.float32)
    return out, res

